# revision 2
# baseline (speedup 1.0000x reference)
"""Trainium2 Bass kernel for the GODEFunc graph-ODE message-passing module.

Math (per batch b):
    xa   = sum_k conv_w[k] * (adj[k] @ x[b]) + conv_b
    W    = (w * clip(d,0,1)) @ w.T
    out  = tanh(0.5*sigmoid(alpha) * xa - 2*x[b] + x[b] @ W + x0[b]*sigmoid(beta))

Sharding: rows (nodes) split across 8 cores; each core computes its
1024-row slice of the output for all batches.  No collectives needed.

Host-side layout: adj is fed per-core TRANSPOSED (adj_t[k, m, r] =
adj[k, row, m]) so the contraction dim m lands on SBUF partitions and
the PE consumes tiles directly as lhsT — no on-device transposes.
x/x0/x-rows/y are fed as [128, chunk, b, f] so every DMA line is
contiguous (>=4KB descriptors).

Per-core kernel structure:
  - adj_t streams in as bf16 (cast during SWDGE DMA), 4MB per DMA.
  - DVE combines k=0,1 with conv_w scalars -> A_eff^T tiles in SBUF.
  - Main matmuls: psum_y[ntt] += A_eff^T_tile.T @ x4[mc] with x4 = all 4
    batches' x rows for chunk mc, resident in SBUF as bf16 [128, (b f)].
  - x @ (W - 2I) is done in fp32 via per-tile PE transposes of x rows.
  - Epilogue: out = tanh(0.5*siga*psum_y + xw + x0*sigmoid(beta) + bias).
"""

import sys

for _p in ("/opt/trn_rl_repo",):
    if _p not in sys.path:
        sys.path.insert(0, _p)

from contextlib import ExitStack

import numpy as np

import concourse.bass as bass
import concourse.mybir as mybir
import concourse.tile as tile
from concourse import bacc
from concourse.bass_utils import run_bass_kernel_spmd
from concourse.masks import make_identity

dt = mybir.dt
AF = mybir.ActivationFunctionType
ALU = mybir.AluOpType

B, N, F, K = 4, 8192, 64, 2
N_CORES = 8
P = 128
GSZ = 8  # m-chunks (of 128) per adj DMA


def build_kernel(n=N, n_cores=N_CORES, b=B, f=F, k_dim=K):
    """Build the per-core Bass module.  All cores run the same program on
    their own row shard."""
    ns = n // n_cores          # rows per core
    nt_cnt = ns // P           # output row tiles per core
    mc_cnt = n // P            # contraction chunks
    ng = mc_cnt // GSZ         # adj DMA groups along contraction dim

    nc = bacc.Bacc(None, target_bir_lowering=False, debug=False)

    adj_t = nc.dram_tensor("adj_t", [k_dim, n, ns], dt.float32, kind="ExternalInput")
    x_t = nc.dram_tensor("x_t", [P, mc_cnt, b, f], dt.float32, kind="ExternalInput")
    xr_t = nc.dram_tensor("xr_t", [P, nt_cnt, b, f], dt.float32, kind="ExternalInput")
    x0_t = nc.dram_tensor("x0_t", [P, nt_cnt, b, f], dt.float32, kind="ExternalInput")
    alpha = nc.dram_tensor("alpha", [ns], dt.float32, kind="ExternalInput")
    beta = nc.dram_tensor("beta", [ns], dt.float32, kind="ExternalInput")
    w = nc.dram_tensor("w", [f, f], dt.float32, kind="ExternalInput")
    d = nc.dram_tensor("d", [f], dt.float32, kind="ExternalInput")
    conv_w = nc.dram_tensor("conv_w", [k_dim], dt.float32, kind="ExternalInput")
    conv_b = nc.dram_tensor("conv_b", [1], dt.float32, kind="ExternalInput")
    y_t = nc.dram_tensor("y_t", [P, nt_cnt, b, f], dt.float32, kind="ExternalOutput")

    bf = b * f  # stacked batch-feature columns

    with tile.TileContext(nc) as tc, ExitStack() as ctx:
        const = ctx.enter_context(tc.tile_pool(name="const", bufs=1))
        xres = ctx.enter_context(tc.tile_pool(name="xres", bufs=1))
        adj_pool = ctx.enter_context(tc.tile_pool(name="adjp", bufs=3))
        work = ctx.enter_context(tc.tile_pool(name="work", bufs=2))
        outp = ctx.enter_context(tc.tile_pool(name="outp", bufs=3))
        keep = ctx.enter_context(tc.tile_pool(name="keep", bufs=1))
        psy = ctx.enter_context(tc.tile_pool(name="psy", bufs=1, space="PSUM"))
        pst_pool = ctx.enter_context(tc.tile_pool(name="pst", bufs=2, space="PSUM"))
        paux = ctx.enter_context(tc.tile_pool(name="paux", bufs=1, space="PSUM"))

        # ---------------- resident x (bf16, all batches, contraction layout) ----
        # x4[g] holds chunks mc = g*GSZ .. (g+1)*GSZ-1; separate tiles so the
        # first matmul group only waits on its own 1MB DMA.
        x4 = []
        for g in range(ng):
            xt = xres.tile([P, GSZ, b, f], dt.bfloat16, tag=f"x4_{g}", name=f"x4_{g}")
            nc.gpsimd.dma_start(out=xt[:], in_=x_t[:, g * GSZ : (g + 1) * GSZ])
            x4.append(xt)

        # ---------------- constants / gates ----------------
        ident_f = const.tile([f, f], dt.float32, tag="ident_f")
        make_identity(nc, ident_f[:])
        ident_p = const.tile([P, P], dt.float32, tag="ident_p")
        make_identity(nc, ident_p[:])

        w_sb = const.tile([f, f], dt.float32, tag="w_sb")
        nc.sync.dma_start(out=w_sb[:], in_=w[:, :])
        d_sb = const.tile([f, 1], dt.float32, tag="d_sb")
        nc.sync.dma_start(out=d_sb[:], in_=d[:, None])
        cw_sb = const.tile([P, k_dim], dt.float32, tag="cw_sb")
        nc.sync.dma_start(out=cw_sb[:], in_=conv_w[None, :].to_broadcast((P, k_dim)))
        cb_sb = const.tile([P, 1], dt.float32, tag="cb_sb")
        nc.sync.dma_start(out=cb_sb[:], in_=conv_b[None, :].to_broadcast((P, 1)))

        al_sb = const.tile([P, nt_cnt], dt.float32, tag="al_sb")
        nc.sync.dma_start(out=al_sb[:], in_=alpha.rearrange("(t p) -> p t", p=P))
        be_sb = const.tile([P, nt_cnt], dt.float32, tag="be_sb")
        nc.sync.dma_start(out=be_sb[:], in_=beta.rearrange("(t p) -> p t", p=P))

        # siga_half[p, nt] = 0.5 * sigmoid(alpha) — row scale for the adj term
        siga = const.tile([P, nt_cnt], dt.float32, tag="siga")
        nc.scalar.activation(siga[:], al_sb[:], AF.Sigmoid)
        siga_half = const.tile([P, nt_cnt], dt.float32, tag="siga_half")
        nc.vector.tensor_scalar(siga_half[:], siga[:], 0.5, None, ALU.mult)
        sigb = const.tile([P, nt_cnt], dt.float32, tag="sigb")
        nc.scalar.activation(sigb[:], be_sb[:], AF.Sigmoid)
        # bias_cb[p, nt] = 0.5 * sigmoid(alpha) * conv_b
        bias_cb = const.tile([P, nt_cnt], dt.float32, tag="bias_cb")
        nc.vector.tensor_scalar(
            bias_cb[:], siga_half[:], cb_sb[:, 0:1], None, ALU.mult
        )

        # ---------------- W' = (w * clip(d,0,1)) @ w.T - 2I ----------------
        pw = paux.tile([f, f], dt.float32, tag="paux")
        nc.tensor.matmul(
            pw[:], w_sb[:], ident_f[:], is_transpose=True, start=True, stop=True
        )
        wT = const.tile([f, f], dt.float32, tag="wT")
        nc.any.tensor_copy(wT[:], pw[:])
        dc = const.tile([f, 1], dt.float32, tag="dc")
        nc.vector.tensor_scalar(dc[:], d_sb[:], 0.0, 1.0, ALU.max, ALU.min)
        wdc = const.tile([f, f], dt.float32, tag="wdc")
        nc.vector.tensor_scalar(wdc[:], wT[:], dc[:], None, ALU.mult)
        pw2 = paux.tile([f, f], dt.float32, tag="paux")
        nc.tensor.matmul(pw2[:], wT[:], wdc[:], start=True, stop=True)
        wp = const.tile([f, f], dt.float32, tag="wp")
        nc.vector.scalar_tensor_tensor(
            wp[:], ident_f[:], -2.0, pw2[:], ALU.mult, ALU.add
        )

        # ---------------- psum accumulators: two row-tiles per bank ----------
        n_banks = (nt_cnt + 1) // 2
        psum_y = [
            psy.tile([P, 2 * bf], dt.float32, tag=f"y{i}", name=f"psum_y{i}")
            for i in range(n_banks)
        ]

        def y_region(ntt):
            return psum_y[ntt // 2][:, (ntt % 2) * bf : (ntt % 2 + 1) * bf]

        # ---------------- xw = x_rows @ (W - 2I), plus x0/beta epilogue prep ----
        xwx0 = []
        for ntt in range(nt_cnt):
            xr = work.tile([P, b, f], dt.float32, tag="xr")
            nc.sync.dma_start(out=xr[:], in_=xr_t[:, ntt])
            pxw = paux.tile([P, bf], dt.float32, tag="paux")
            for bb in range(b):
                pxT = pst_pool.tile([f, P], dt.float32, tag="pst")
                nc.tensor.matmul(
                    pxT[:], xr[:, bb, :], ident_p[:],
                    is_transpose=True, start=True, stop=True,
                )
                xT = work.tile([f, P], dt.float32, tag="xT")
                nc.any.tensor_copy(xT[:], pxT[:])
                nc.tensor.matmul(
                    pxw[:, bb * f : (bb + 1) * f], xT[:], wp[:],
                    start=True, stop=True,
                )
            x0t = work.tile([P, b, f], dt.float32, tag="x0t")
            nc.sync.dma_start(out=x0t[:], in_=x0_t[:, ntt])
            acc = keep.tile([P, bf], dt.float32, tag=f"xwx0_{ntt}")
            # acc = x0 * sigmoid(beta) + xw
            nc.vector.scalar_tensor_tensor(
                acc[:],
                x0t[:].rearrange("p b f -> p (b f)"),
                sigb[:, ntt : ntt + 1],
                pxw[:],
                ALU.mult,
                ALU.add,
            )
            xwx0.append(acc)

        # ---------------- main loop: stream adj_t, combine k, matmul ------
        for mg in range(ng):
            rows_m = slice(mg * GSZ * P, (mg + 1) * GSZ * P)
            a_tiles = []
            for kk in range(k_dim):
                a_k = adj_pool.tile(
                    [P, GSZ, ns], dt.bfloat16, tag=f"adj{kk}", name=f"adj_t{kk}"
                )
                nc.gpsimd.dma_start(
                    out=a_k[:],
                    in_=adj_t[kk, rows_m, :].rearrange("(g p) r -> p g r", p=P),
                )
                a_tiles.append(a_k)
            # a0 <- sum_k conv_w[k] * a_k  (plain scalar weights; the row
            # gate 0.5*sigmoid(alpha) is applied in the epilogue)
            nc.vector.tensor_scalar(
                a_tiles[1][:], a_tiles[1][:], cw_sb[:, 1:2], None, ALU.mult
            )
            nc.vector.scalar_tensor_tensor(
                a_tiles[0][:], a_tiles[0][:], cw_sb[:, 0:1], a_tiles[1][:],
                ALU.mult, ALU.add,
            )
            for g in range(GSZ):
                mc = mg * GSZ + g
                for ntt in range(nt_cnt):
                    nc.tensor.matmul(
                        y_region(ntt),
                        a_tiles[0][:, g, ntt * P : (ntt + 1) * P],
                        x4[mg][:, g],
                        start=(mc == 0),
                        stop=(mc == mc_cnt - 1),
                        skip_group_check=True,
                    )

        # ---------------- epilogue: tanh(0.5*siga*psum_y + xwx0 + bias) ------
        for ntt in range(nt_cnt):
            acc = outp.tile([P, bf], dt.float32, tag="eacc")
            nc.vector.scalar_tensor_tensor(
                acc[:], y_region(ntt), siga_half[:, ntt : ntt + 1], xwx0[ntt][:],
                ALU.mult, ALU.add,
            )
            outt = outp.tile([P, bf], dt.float32, tag="outt")
            nc.scalar.activation(
                outt[:], acc[:], AF.Tanh, bias=bias_cb[:, ntt : ntt + 1]
            )
            nc.sync.dma_start(
                out=y_t[:, ntt],
                in_=outt[:].rearrange("p (b f) -> p b f", b=b),
            )

    nc.finalize()
    return nc


_NC_CACHE = {}


def _get_nc(key=(N, N_CORES, B, F, K)):
    if key not in _NC_CACHE:
        _NC_CACHE[key] = build_kernel(*key)
    return _NC_CACHE[key]


def make_in_maps(x, x0, adj, alpha, beta, w, d, conv_w, conv_b, n_cores=N_CORES):
    """Slice + re-lay the full inputs into per-core shards."""
    n = x.shape[1]
    ns = n // n_cores
    b, f = x.shape[0], x.shape[2]
    nt = ns // P
    mc = n // P
    f32 = np.float32

    # x_t[p, mc, b, f] = x[b, mc*128+p, f] — shared by all cores
    x_t = np.ascontiguousarray(
        x.reshape(b, mc, P, f).transpose(2, 1, 0, 3), dtype=f32
    )

    in_maps = []
    for c in range(n_cores):
        rows = slice(c * ns, (c + 1) * ns)
        adj_tc = np.ascontiguousarray(
            adj[:, rows, :].swapaxes(1, 2), dtype=f32
        )
        x0_tc = np.ascontiguousarray(
            x0[:, rows, :].reshape(b, nt, P, f).transpose(2, 1, 0, 3), dtype=f32
        )
        xr_tc = np.ascontiguousarray(x_t[:, c * nt : (c + 1) * nt], dtype=f32)
        in_maps.append(
            {
                "adj_t": adj_tc,
                "x_t": x_t,
                "xr_t": xr_tc,
                "x0_t": x0_tc,
                "alpha": np.ascontiguousarray(alpha[rows], dtype=f32),
                "beta": np.ascontiguousarray(beta[rows], dtype=f32),
                "w": np.ascontiguousarray(w, dtype=f32),
                "d": np.ascontiguousarray(d, dtype=f32),
                "conv_w": np.ascontiguousarray(conv_w, dtype=f32),
                "conv_b": np.ascontiguousarray(conv_b, dtype=f32),
            }
        )
    return in_maps


def kernel(x, x0, adj, alpha, beta, w, d, conv_w, conv_b):
    x = np.asarray(x)
    x0 = np.asarray(x0)
    adj = np.asarray(adj)
    alpha = np.asarray(alpha)
    beta = np.asarray(beta)
    w = np.asarray(w)
    d = np.asarray(d)
    conv_w = np.asarray(conv_w)
    conv_b = np.asarray(conv_b)

    b, n, f = x.shape
    ns = n // N_CORES
    nt = ns // P

    nc = _get_nc()
    in_maps = make_in_maps(x, x0, adj, alpha, beta, w, d, conv_w, conv_b)
    res = run_bass_kernel_spmd(nc, in_maps, core_ids=list(range(N_CORES)))
    # y_t[p, nt, b, f] -> y[b, c*ns + nt*128 + p, f]
    parts = [
        res.results[c]["y_t"].transpose(2, 1, 0, 3).reshape(b, ns, f)
        for c in range(N_CORES)
    ]
    out = np.concatenate(parts, axis=1)
    return out.astype(np.float32)
